# revision 35
# baseline (speedup 1.0000x reference)
# GAT (graph attention) Trainium2 kernel — 8-core row-parallel SPMD.
#
# Math (per head h, rows I owned by a core):
#   h = x @ W_h ; f1 = h@a1 ; f2 = h@a2 ; z_ij = f1_i + f2_j
#   P_ij = adj_ij ? exp(lrelu(z)) : exp(9e-15 ~= 0) ; att = softmax_j(P)
#   out = elu( (P @ h) / (P @ 1) )
# Device factorization (avoids O(N^2) transcendentals):
#   exp(lrelu(z)) = u'_i * v'_j * max(r_i * w_j, 1)
#     r = e^{0.8 f1}, w = e^{0.8 f2}, u' = e^{0.2 f1}, v' = e^{0.2 f2}
#   E2[j,i] = m^T[j,i] * max(r_i * (w_j v'_j), v'_j)     (ts_dual + tt mult)
#   numer[d,i] = u'_i * ([h|1]^T E2)[d,i] + S[d] - (h^T m^T)[d,i]
#   denom[i]   = u'_i * Y1[i] + N - deg_i
#
# Host->device traffic is the e2e bottleneck (slow axon RPC link), so:
#   - adj ships BIT-PACKED along j (uint8 [nb, n/8], 32x less wire) and is
#     unpacked on device with an is_ge/subtract cascade (float-only ALU ops).
#     Unpacked planes live at j' = b*(n/8)+k  <->  original j = 8k+b.
#   - x (bf16), W (bf16) and a (f32) ship only as per-core slices in one
#     combined byte buffer; the full tensors are reassembled on device with
#     a single 8-core AllGather, then upcast to f32 via SWDGE cast-DMAs.
#     The bitplane j-permutation is applied when loading x^T chunks
#     (stride-8 row reads of the gathered x).
#   - output returns int8 with per-row f32 scales bitcast into tail rows of
#     the same tensor (one RPC get); host dequantizes to f32.

import numpy as np

N = 8192
EMB = 128
HID = 64
NH = 4
NCORES = 8
NB = N // NCORES  # 1024 rows per core
NP8 = N // 8      # packed bytes per adj row

_cache = {}


def build(n=N, nb=NB):
    import concourse.bass as bass
    import concourse.bacc as bacc
    import concourse.tile as tile
    import concourse.mybir as mybir
    from concourse.masks import make_identity

    fp32 = mybir.dt.float32
    bf16 = mybir.dt.bfloat16
    u8 = mybir.dt.uint8
    Alu = mybir.AluOpType
    Act = mybir.ActivationFunctionType
    MS = bass.MemorySpace

    nbh = nb // 2           # i-half size
    njc = n // 128          # j chunks
    nic = nb // 128         # i chunks (local rows)
    nsub = nbh // 128       # i subchunks per half
    np8 = n // 8            # packed bytes per row

    nc = bacc.Bacc(num_devices=NCORES)
    # One combined small-input param per core (fewer RPC puts on the slow
    # axon link): [ x_blk bf16 | W-eighth bf16 | a-eighth f32 ] as raw bytes.
    # W and a ship as per-core eighths and ride the x AllGather instead of
    # being host-replicated 8x.
    xb_bytes = nb * EMB * 2
    w8_bytes = NH * EMB * HID * 2 // NCORES
    a8_bytes = NH * 2 * HID * 4 // NCORES
    seg = xb_bytes + w8_bytes + a8_bytes
    xwa_d = nc.declare_dram_parameter("xwa_blk", [seg], u8, isOutput=False)
    adjp_d = nc.declare_dram_parameter("adjp_blk", [nb, np8], u8, isOutput=False)
    xb_d = xwa_d[0:xb_bytes].bitcast(bf16).rearrange("(r c) -> r c", r=nb)
    # int8 output with per-row f32 scales bitcast into 16 tail rows (one
    # fetched tensor — a second RPC get costs ~80ms fixed on the axon link).
    i8 = mybir.dt.int8
    out_d = nc.declare_dram_parameter(
        "out_blk", [nb + (nb // 128) * 2, NH * HID], i8, isOutput=True)

    with tile.TileContext(nc) as tc:
        with (
            tc.tile_pool(name="const", bufs=1) as const,
            tc.tile_pool(name="ld", bufs=3) as ld,
            tc.tile_pool(name="anat", bufs=2) as anat,
            tc.tile_pool(name="dramp", bufs=1, space=MS.DRAM) as dramp,
            tc.tile_pool(name="mtp", bufs=6) as mtp,
            tc.tile_pool(name="dep", bufs=8) as dep,
            tc.tile_pool(name="esb", bufs=3) as esb,
        ):
            # ------- all-gather [x | W/8 | a/8] (bf16 wire, f32 on device) ---
            ag_in = dramp.tile([seg], u8, name="ag_in", tag="ag_in")
            ag_out = dramp.tile([NCORES * seg], u8, name="ag_out", tag="ag_out",
                                addr_space="Shared")
            nc.sync.dma_start(out=ag_in, in_=xwa_d[:])
            nc.gpsimd.collective_compute(
                "AllGather", Alu.bypass,
                replica_groups=[list(range(NCORES))],
                ins=[ag_in], outs=[ag_out],
            )
            g2 = ag_out.rearrange("(c y) -> c y", c=NCORES)
            # one SWDGE cast pass each: gathered x and the local block -> f32
            xag = dramp.tile([n, EMB], fp32, name="xag", tag="xag")
            nc.gpsimd.dma_start(out=xag, in_=g2[:, 0:xb_bytes].bitcast(bf16))
            xb_f = dramp.tile([nb, EMB], fp32, name="xb_f", tag="xb_f")
            nc.gpsimd.dma_start(out=xb_f, in_=xb_d)
            # reassemble W (bf16) and a (f32) from the gathered eighths
            Wg = dramp.tile([NH * EMB * HID], bf16, name="Wg", tag="Wg")
            nc.sync.dma_start(
                out=Wg, in_=g2[:, xb_bytes:xb_bytes + w8_bytes].bitcast(bf16))
            ag_a = dramp.tile([NH * 2 * HID], fp32, name="ag_a", tag="ag_a")
            nc.sync.dma_start(
                out=ag_a, in_=g2[:, xb_bytes + w8_bytes:seg].bitcast(fp32))
            W_v = Wg.rearrange("(h e d) -> e h d", h=NH, e=EMB)
            a_v = ag_a.rearrange("(h t d o) -> d h (t o)", h=NH, t=2, o=1)
            # permuted row view: row (b k) of xagv == original row 8k+b == j'
            xagv = xag.rearrange("(k b) e -> b k e", b=8)

            # ---------------- constants ----------------
            ident = const.tile([128, 128], fp32, name="ident", tag="ident")
            make_identity(nc, ident)
            ones_row = const.tile([1, 128], fp32, name="ones_row", tag="ones_row")
            nc.vector.memset(ones_row, 1.0)

            madj0 = dramp.tile([nbh, n], bf16, name="madj0", tag="madj0")
            madj1 = dramp.tile([nbh, n], bf16, name="madj1", tag="madj1")
            madj = [madj0, madj1]
            deg_sb = const.tile([128, nic], fp32, name="deg_sb", tag="deg_sb")

            # ------------- stage A: bit-unpack mask (+ deg) -------------
            # planes b=7..0 via is_ge cascade; plane b lands at j' cols
            # [b*np8, (b+1)*np8) matching x_perm row order from the allgather.
            def stageA(ihalf):
                for ics in range(nic // 2):
                    r0 = ihalf * nbh + ics * 128
                    icg = ihalf * (nic // 2) + ics
                    v = [anat.tile([128, np8], fp32, name=f"v{i}", tag=f"v{i}")
                         for i in range(2)]
                    nc.gpsimd.dma_start(out=v[0], in_=adjp_d[r0:r0 + 128, :])
                    dacc = anat.tile([128, 8], fp32, name="dacc", tag="dacc")
                    for s, b in enumerate(range(7, -1, -1)):
                        pbf = anat.tile([128, np8], bf16, name=f"pbf{b}",
                                        tag="pbf")
                        nc.vector.tensor_scalar(
                            out=pbf, in0=v[s % 2],
                            scalar1=float(1 << b), scalar2=0.0,
                            op0=Alu.is_ge, op1=Alu.add,
                            accum_out=dacc[:, b:b + 1])
                        if b > 0:
                            nc.vector.scalar_tensor_tensor(
                                out=v[(s + 1) % 2], in0=pbf,
                                scalar=-float(1 << b), in1=v[s % 2],
                                op0=Alu.mult, op1=Alu.add)
                        nc.sync.dma_start(
                            out=madj[ihalf][ics * 128:(ics + 1) * 128,
                                            b * np8:(b + 1) * np8],
                            in_=pbf)
                    nc.vector.tensor_reduce(
                        deg_sb[:, icg:icg + 1], dacc,
                        mybir.AxisListType.X, Alu.add)

            stageA(0)

            # ---------------- prologue ----------------
            ppsum = tc.alloc_tile_pool(name="ppsum", bufs=2, space=MS.PSUM)
            Wsb = const.tile([128, NH, HID], fp32, name="Wsb", tag="Wsb")
            nc.gpsimd.dma_start(out=Wsb, in_=W_v)   # bf16 wire -> f32 compute
            asb = const.tile([HID, NH, 2], fp32, name="asb", tag="asb")
            nc.sync.dma_start(out=asb, in_=a_v)

            # x_perm^T  [128e, n] — chunk jc holds j' = jc*128 .. (jc+1)*128
            xT = const.tile([128, n], fp32, name="xT", tag="xT")
            for jc in range(njc):
                b, m = jc // 8, jc % 8
                xt_nat = ld.tile([128, EMB], fp32, name="xt_nat", tag="xt_nat")
                nc.sync.dma_start(
                    out=xt_nat, in_=xagv[b, m * 128:(m + 1) * 128, :])
                ps = ppsum.tile([128, 128], fp32, name="ps", tag="ps")
                nc.tensor.matmul(ps, xt_nat, ident)
                nc.scalar.copy(out=xT[:, jc * 128:(jc + 1) * 128], in_=ps)
            # x_blk^T [128e, nb]
            xbT = const.tile([128, nb], fp32, name="xbT", tag="xbT")
            for ic in range(nic):
                xb_nat = ld.tile([128, EMB], fp32, name="xb_nat", tag="xt_nat")
                nc.sync.dma_start(out=xb_nat, in_=xb_f[ic * 128:(ic + 1) * 128, :])
                ps = ppsum.tile([128, 128], fp32, name="ps", tag="ps")
                nc.tensor.matmul(ps, xb_nat, ident)
                nc.scalar.copy(out=xbT[:, ic * 128:(ic + 1) * 128], in_=ps)

            # xsum[e] = sum_j x[j,e]
            xsum = const.tile([128, 1], fp32, name="xsum", tag="xsum")
            nc.vector.tensor_reduce(xsum, xT, mybir.AxisListType.X, Alu.add)

            # W^T per head; q = [W a1 | W a2] -> Qsb [128e, NH, 2]
            WTsb = const.tile([HID, NH, 128], fp32, name="WTsb", tag="WTsb")
            Qsb = const.tile([128, NH, 2], fp32, name="Qsb", tag="Qsb")
            for h in range(NH):
                wt_ps = ppsum.tile([HID, 128], fp32, name="wt_ps", tag="ps")
                nc.tensor.matmul(wt_ps, Wsb[:, h, :], ident)
                nc.scalar.copy(out=WTsb[:, h, :], in_=wt_ps)
                q_ps = ppsum.tile([128, 2], fp32, name="q_ps", tag="ps")
                nc.tensor.matmul(q_ps, WTsb[:, h, :], asb[:, h, :])
                nc.scalar.copy(out=Qsb[:, h, :], in_=q_ps)

            Qflat = Qsb.rearrange("p h t -> p (h t)")
            Wflat = Wsb.rearrange("e h d -> e (h d)")

            # f columns for all j: Fcol[p, jc, (h t)] = f_{t,h}[jc*128+p]
            Fcol = const.tile([128, njc, 2 * NH], fp32, name="Fcol", tag="Fcol")
            for jc in range(njc):
                f_ps = ppsum.tile([128, 2 * NH], fp32, name="f_ps", tag="ps")
                nc.tensor.matmul(f_ps, xT[:, jc * 128:(jc + 1) * 128], Qflat)
                nc.scalar.copy(out=Fcol[:, jc, :], in_=f_ps)

            # f rows for local block: Frow [8, nb]
            Frow = const.tile([2 * NH, nb], fp32, name="Frow", tag="Frow")
            for half in range(2):
                fr_ps = ppsum.tile([2 * NH, nbh], fp32, name="fr_ps", tag="ps")
                nc.tensor.matmul(fr_ps, Qflat, xbT[:, half * nbh:(half + 1) * nbh])
                nc.scalar.copy(out=Frow[:, half * nbh:(half + 1) * nbh], in_=fr_ps)

            # FrowT [128, nic, 8]
            FrowT = const.tile([128, nic, 2 * NH], fp32, name="FrowT", tag="FrowT")
            for g in range(nic):
                ft_ps = ppsum.tile([128, 2 * NH], fp32, name="ft_ps", tag="ps")
                nc.tensor.matmul(
                    ft_ps, Frow[:, g * 128:(g + 1) * 128],
                    ident[0:2 * NH, 0:2 * NH])
                nc.scalar.copy(out=FrowT[:, g, :], in_=ft_ps)

            # scalar cols (j side): ETc = e^{f2} (= w v'), Vc = e^{0.2 f2}
            ETc = const.tile([128, njc, NH], fp32, name="ETc", tag="ETc")
            Vc = const.tile([128, njc, NH], fp32, name="Vc", tag="Vc")
            for h in range(NH):
                nc.scalar.activation(ETc[:, :, h], Fcol[:, :, 2 * h + 1], Act.Exp)
                nc.scalar.activation(
                    Vc[:, :, h], Fcol[:, :, 2 * h + 1], Act.Exp, scale=0.2)

            # row side: R8 = e^{0.8 Frow}; U'T = e^{0.2 FrowT}
            R8 = const.tile([2 * NH, nb], fp32, name="R8", tag="R8")
            nc.scalar.activation(R8, Frow, Act.Exp, scale=0.8)
            UpT = const.tile([128, nic, 2 * NH], fp32, name="UpT", tag="UpT")
            nc.scalar.activation(
                UpT.rearrange("p a b -> p (a b)"),
                FrowT.rearrange("p a b -> p (a b)"), Act.Exp, scale=0.2)

            # r broadcast per head [128, nb] bf16: bounce rows via DRAM, then
            # broadcast-load with stride-0 partition AP (+ cast) via SWDGE.
            r8_dram = dramp.tile([2 * NH, nb], fp32, name="r8_dram", tag="r8d")
            nc.sync.dma_start(out=r8_dram, in_=R8)
            rbc = []
            for h in range(NH):
                t = const.tile([128, nb], bf16, name=f"rbc{h}", tag=f"rbc{h}")
                srow = r8_dram[2 * h:2 * h + 1, :]
                src_b = bass.AP(
                    tensor=srow.tensor, offset=srow.offset,
                    ap=[[0, 128]] + [list(d) for d in srow.ap[1:]])
                nc.gpsimd.dma_start(out=t, in_=src_b)
                rbc.append(t)

            # H~ [128, njc, NH, HID+1] bf16 (ones col at [.., HID]) for the
            # per-head X passes, plus a contiguous pair layout for hm passes
            # (matmul weights APs must have a single free dimension).
            Hsb = const.tile([128, njc, NH, HID + 1], bf16, name="Hsb", tag="Hsb")
            Hpair = const.tile([128, njc, NH * HID], bf16, name="Hpair", tag="Hpair")
            nc.vector.memset(Hsb[:, :, :, HID], 1.0)
            for jc in range(njc):
                h_ps = ppsum.tile([128, NH, HID], fp32, name="h_ps", tag="ps")
                nc.tensor.matmul(
                    h_ps.rearrange("p h d -> p (h d)"),
                    xT[:, jc * 128:(jc + 1) * 128], Wflat)
                nc.scalar.copy(out=Hsb[:, jc, :, 0:HID], in_=h_ps)
                nc.scalar.copy(
                    out=Hpair[:, jc, :].rearrange("p (h d) -> p h d", h=NH),
                    in_=h_ps)

            # S row then per-head broadcast [128, HID]
            s_ps = ppsum.tile([1, NH * HID], fp32, name="s_ps", tag="ps")
            nc.tensor.matmul(s_ps, xsum, Wflat)
            S_row = const.tile([1, NH * HID], fp32, name="S_row", tag="S_row")
            nc.scalar.copy(out=S_row, in_=s_ps)
            Sb = []
            for h in range(NH):
                sb_ps = ppsum.tile([128, HID], fp32, name="sb_ps", tag="ps")
                nc.tensor.matmul(sb_ps, ones_row, S_row[:, h * HID:(h + 1) * HID])
                t = const.tile([128, HID], fp32, name=f"Sb{h}", tag=f"Sb{h}")
                nc.scalar.copy(out=t, in_=sb_ps)
                Sb.append(t)

            ppsum.release()

            # ------------- stage A part 2, then degbar -------------
            stageA(1)
            degbar = const.tile([128, nic], fp32, name="degbar", tag="degbar")
            nc.vector.tensor_scalar(
                out=degbar, in0=deg_sb, scalar1=-1.0, scalar2=float(n),
                op0=Alu.mult, op1=Alu.add)

            # ---------------- main loop ----------------
            for ihalf in range(2):
                with (
                    tc.tile_pool(name=f"mm{ihalf}", bufs=1, space=MS.PSUM) as mm,
                    tc.tile_pool(name=f"ep{ihalf}", bufs=2, space=MS.PSUM) as ep,
                ):
                    X = [mm.tile([HID + 1, nbh], fp32, name=f"X{h}", tag=f"X{h}")
                         for h in range(NH)]
                    HM = [mm.tile([128, nbh], fp32, name=f"HM{p}", tag=f"HM{p}")
                          for p in range(2)]
                    for jc in range(njc):
                        mT = mtp.tile([128, nbh], bf16, name="mT", tag="mT")
                        nc.sync.dma_start_transpose(
                            out=mT,
                            in_=madj[ihalf][:, jc * 128:(jc + 1) * 128])
                        for h in range(NH):
                            D2 = dep.tile([128, nbh], bf16, name="D2", tag="D2")
                            nc.vector.tensor_scalar(
                                out=D2,
                                in0=rbc[h][:, ihalf * nbh:(ihalf + 1) * nbh],
                                scalar1=ETc[:, jc, h:h + 1],
                                scalar2=Vc[:, jc, h:h + 1],
                                op0=Alu.mult, op1=Alu.max)
                            E2 = dep.tile([128, nbh], bf16, name="E2", tag="E2")
                            eng_tt = nc.gpsimd if h >= 2 else nc.vector
                            eng_tt.tensor_mul(E2, mT, D2)
                            nc.tensor.matmul(
                                X[h], Hsb[:, jc, h, :], E2,
                                start=(jc == 0), stop=(jc == njc - 1))
                        for p in range(2):
                            nc.tensor.matmul(
                                HM[p],
                                Hpair[:, jc, 128 * p:128 * (p + 1)], mT,
                                start=(jc == 0), stop=(jc == njc - 1))

                    # ---------------- epilogue for this half ----------------
                    XS = []
                    for h in range(NH):
                        t = esb.tile([HID + 1, nbh], fp32,
                                     name=f"XS{h}", tag=f"XS{h}", bufs=1)
                        nc.scalar.copy(out=t, in_=X[h])
                        XS.append(t)
                    HMS = []
                    for p in range(2):
                        t = esb.tile([128, nbh], fp32,
                                     name=f"HMS{p}", tag=f"HMS{p}", bufs=1)
                        nc.scalar.copy(out=t, in_=HM[p])
                        HMS.append(t)

                    for isub in range(nsub):
                        g = ihalf * nsub + isub
                        sl = slice(isub * 128, (isub + 1) * 128)
                        hmT = []
                        for p in range(2):
                            tp = ep.tile([128, 128], fp32, name="tp", tag="tp")
                            nc.tensor.matmul(tp, HMS[p][:, sl], ident)
                            t = esb.tile([128, 128], fp32,
                                         name=f"hmT{p}", tag=f"hmT{p}", bufs=2)
                            nc.scalar.copy(out=t, in_=tp)
                            hmT.append(t)
                        out_tile = esb.tile([128, NH * HID], fp32,
                                            name="out_tile", tag="otile", bufs=2)
                        for h in range(NH):
                            tp = ep.tile([128, HID + 1], fp32, name="tpx", tag="tp")
                            nc.tensor.matmul(
                                tp, XS[h][:, sl], ident[0:HID + 1, 0:HID + 1])
                            XT = esb.tile([128, HID + 1], fp32, name="XT", tag="XT")
                            nc.scalar.copy(out=XT, in_=tp)
                            upc = UpT[:, g, 2 * h:2 * h + 1]
                            n1 = esb.tile([128, HID], fp32, name="n1", tag="n1")
                            nc.vector.tensor_scalar(
                                out=n1, in0=XT[:, 0:HID], scalar1=upc,
                                scalar2=None, op0=Alu.mult)
                            n2 = esb.tile([128, HID], fp32, name="n2", tag="n2")
                            nc.vector.scalar_tensor_tensor(
                                out=n2,
                                in0=hmT[h // 2][:, (h % 2) * HID:
                                                (h % 2) * HID + HID],
                                scalar=-1.0, in1=n1, op0=Alu.mult, op1=Alu.add)
                            n3 = esb.tile([128, HID], fp32, name="n3", tag="n3")
                            nc.vector.tensor_add(n3, n2, Sb[h])
                            dcol = esb.tile([128, 1], fp32, name="dcol", tag="dcol")
                            nc.vector.tensor_scalar(
                                out=dcol, in0=XT[:, HID:HID + 1], scalar1=upc,
                                scalar2=degbar[:, g:g + 1],
                                op0=Alu.mult, op1=Alu.add)
                            rec = esb.tile([128, 1], fp32, name="rec", tag="rec")
                            nc.vector.reciprocal(rec, dcol)
                            smT = esb.tile([128, HID], fp32, name="smT", tag="smT")
                            nc.vector.tensor_scalar(
                                out=smT, in0=n3, scalar1=rec, scalar2=None,
                                op0=Alu.mult)
                            # elu = (max(sm,0)-1) + exp(min(sm,0))
                            ea = esb.tile([128, HID], fp32, name="ea", tag="ea")
                            nc.vector.tensor_scalar_min(ea, smT, 0.0)
                            eb = esb.tile([128, HID], fp32, name="eb", tag="eb")
                            nc.scalar.activation(eb, ea, Act.Exp)
                            ec = esb.tile([128, HID], fp32, name="ec", tag="ec")
                            nc.vector.tensor_scalar(
                                out=ec, in0=smT, scalar1=0.0, scalar2=-1.0,
                                op0=Alu.max, op1=Alu.add)
                            nc.vector.tensor_add(
                                out_tile[:, h * HID:(h + 1) * HID], eb, ec)
                        # int8 quantize with per-row scale (host dequantizes):
                        # scale = max(|row|)/127, q = round(row/scale)
                        rhi = esb.tile([128, 1], fp32, name="rhi", tag="rhi")
                        nc.vector.tensor_reduce(
                            rhi, out_tile, mybir.AxisListType.X, Alu.max)
                        rlo = esb.tile([128, 1], fp32, name="rlo", tag="rlo")
                        nc.vector.tensor_reduce(
                            rlo, out_tile, mybir.AxisListType.X, Alu.min)
                        rneg = esb.tile([128, 1], fp32, name="rneg", tag="rneg")
                        nc.vector.tensor_scalar(
                            out=rneg, in0=rlo, scalar1=-1.0, scalar2=None,
                            op0=Alu.mult)
                        rabs = esb.tile([128, 1], fp32, name="rabs", tag="rabs")
                        nc.vector.tensor_tensor(rabs, rhi, rneg, Alu.max)
                        rsc = esb.tile([128, 1], fp32, name="rsc", tag="rsc")
                        nc.vector.tensor_scalar(
                            out=rsc, in0=rabs, scalar1=1e-30,
                            scalar2=1.0 / 127.0, op0=Alu.max, op1=Alu.mult)
                        rinv = esb.tile([128, 1], fp32, name="rinv", tag="rinv")
                        nc.vector.reciprocal(rinv, rsc)
                        qt = esb.tile([128, NH * HID], i8,
                                      name="qt", tag="qt", bufs=2)
                        nc.vector.tensor_scalar(
                            out=qt, in0=out_tile, scalar1=rinv, scalar2=None,
                            op0=Alu.mult)
                        nc.sync.dma_start(
                            out=out_d[g * 128:(g + 1) * 128, :], in_=qt)
                        nc.sync.dma_start(
                            out=out_d[nb + 2 * g:nb + 2 * g + 2, :].bitcast(fp32),
                            in_=rsc)
    nc.compile()
    return nc


def _get_nc():
    if "nc" not in _cache:
        _cache["nc"] = build()
    return _cache["nc"]


def _get_runner():
    # Build the 8-core shard_map executable ONCE and reuse it: the stock
    # run_bass_kernel_spmd path re-runs jax.jit(shard_map(...)) per call,
    # which costs ~0.5s of retracing on every invocation. Donated output
    # buffers are created on-device (jit zeros) so they cost no host->device
    # wire traffic.
    if "runner" in _cache:
        return _cache["runner"]
    import jax
    import jax.numpy as jnp
    from jax.sharding import Mesh, PartitionSpec, NamedSharding
    from jax.experimental.shard_map import shard_map
    from concourse import bass2jax, mybir

    nc = _get_nc()
    bass2jax.install_neuronx_cc_hook()
    partition_name = nc.partition_id_tensor.name if nc.partition_id_tensor else None
    in_names, out_names, out_avals = [], [], []
    for alloc in nc.m.functions[0].allocations:
        if not isinstance(alloc, mybir.MemoryLocationSet):
            continue
        name = alloc.memorylocations[0].name
        if alloc.kind == "ExternalInput":
            if name != partition_name:
                in_names.append(name)
        elif alloc.kind == "ExternalOutput":
            out_names.append(name)
            out_avals.append(jax.core.ShapedArray(
                tuple(alloc.tensor_shape), mybir.dt.np(alloc.dtype)))
    n_params = len(in_names)
    n_outs = len(out_avals)
    in_names = in_names + out_names
    if partition_name is not None:
        in_names.append(partition_name)
    donate = tuple(range(n_params, n_params + n_outs))

    def _body(*args):
        operands = list(args)
        if partition_name is not None:
            operands.append(bass2jax.partition_id_tensor())
        outs = bass2jax._bass_exec_p.bind(
            *operands, out_avals=tuple(out_avals), in_names=tuple(in_names),
            out_names=tuple(out_names), lowering_input_output_aliases=(),
            sim_require_finite=True, sim_require_nnan=True, nc=nc)
        return tuple(outs)

    devices = jax.devices()[:NCORES]
    mesh = Mesh(np.asarray(devices), ("core",))
    specs = (PartitionSpec("core"),) * (n_params + n_outs)
    out_specs = (PartitionSpec("core"),) * n_outs
    sharded = jax.jit(
        shard_map(_body, mesh=mesh, in_specs=specs, out_specs=out_specs,
                  check_rep=False),
        donate_argnums=donate, keep_unused=True)

    shard8 = NamedSharding(mesh, PartitionSpec("core"))
    zero_shapes = [(NCORES * av.shape[0], *av.shape[1:]) for av in out_avals]
    zero_dtypes = [av.dtype for av in out_avals]

    def _zeros():
        return tuple(jnp.zeros(s, d) for s, d in zip(zero_shapes, zero_dtypes))

    zeros_jit = jax.jit(_zeros, out_shardings=(shard8,) * n_outs)

    runner = (sharded, zeros_jit, in_names[:n_params], out_names,
              devices, shard8)
    _cache["runner"] = runner
    return runner


def kernel(x, adj, W, a):
    import sys
    for p in ("/opt/trn_rl_repo", "/opt/trn_rl_repo/concourse"):
        if p not in sys.path:
            sys.path.insert(0, p)

    import jax

    x = np.ascontiguousarray(np.asarray(x, dtype=np.float32))
    adj = np.asarray(adj, dtype=np.int32)
    W = np.ascontiguousarray(np.asarray(W, dtype=np.float32))
    a = np.ascontiguousarray(np.asarray(a, dtype=np.float32))

    sharded, zeros_jit, param_names, out_names, devices, sh8 = _get_runner()

    import ml_dtypes

    # Ship the combined [x_blk bf16 | W-eighth bf16 | a-eighth f32] byte
    # buffer per core, putting each slice as soon as it is built so the wire
    # starts moving immediately; the on-device zero output buffers follow.
    # All of this overlaps the host-side adjacency bit-packing below.
    XB = N // NCORES * EMB * 2
    WB8 = NH * EMB * HID * 2 // NCORES
    AB8 = NH * 2 * HID * 4 // NCORES
    seg = XB + WB8 + AB8
    W_bytes = W.astype(ml_dtypes.bfloat16).reshape(-1).view(np.uint8)
    a_bytes = a.reshape(-1).view(np.uint8)
    xwa_rows = []
    xwa_shards = []
    for c in range(NCORES):
        row = np.empty(seg, np.uint8)
        row[:XB] = (x[c * NB:(c + 1) * NB].astype(ml_dtypes.bfloat16)
                    .reshape(-1).view(np.uint8))
        row[XB:XB + WB8] = W_bytes[c * WB8:(c + 1) * WB8]
        row[XB + WB8:] = a_bytes[c * AB8:(c + 1) * AB8]
        xwa_rows.append(row)
        xwa_shards.append(jax.device_put(row, devices[c]))
    xwad = jax.make_array_from_single_device_arrays(
        (NCORES * seg,), sh8, xwa_shards)
    zs = zeros_jit()

    # bit-pack adjacency along j: adj is 0/1 int32 (little-endian), so the
    # low byte of each int32 is the value; packbits treats nonzero as 1.
    # Pack per core block and start each block's device transfer immediately
    # so wire time pipelines with the packing of later blocks.
    if adj.flags.c_contiguous:
        adj_bytes = adj.view(np.uint8)[:, ::4]
    else:
        adj_bytes = adj.astype(np.uint8)
    shards = []
    for c in range(NCORES):
        blk = np.packbits(adj_bytes[c * NB:(c + 1) * NB], axis=1,
                          bitorder="little")           # [NB, N/8] u8
        shards.append(jax.device_put(blk, devices[c]))
    adjp_arr = jax.make_array_from_single_device_arrays(
        (N, NP8), sh8, shards)

    vals = {"xwa_blk": xwad, "adjp_blk": adjp_arr}
    try:
        out_arrs = sharded(*[vals[nm] for nm in param_names], *zs)
        # fetching right after dispatch overlaps the execute-completion wait
        # with the output transfer
        out_arrs[0].copy_to_host_async()
        arr = np.asarray(out_arrs[0])    # [NCORES*(NB+16), NH*HID] int8
    except Exception:
        # transient axon/NRT hiccup (e.g. a stale exec unit from a prior
        # session) — re-put everything once and retry
        xwad = jax.make_array_from_single_device_arrays(
            (NCORES * seg,), sh8,
            [jax.device_put(r, devices[c]) for c, r in enumerate(xwa_rows)])
        shards = [jax.device_put(np.packbits(
            adj_bytes[c * NB:(c + 1) * NB], axis=1, bitorder="little"),
            devices[c]) for c in range(NCORES)]
        adjp_arr = jax.make_array_from_single_device_arrays((N, NP8), sh8, shards)
        vals = {"xwa_blk": xwad, "adjp_blk": adjp_arr}
        out_arrs = sharded(*[vals[nm] for nm in param_names], *zeros_jit())
        arr = np.asarray(out_arrs[0])
    nbp = NB + (NB // 128) * 2
    blocks = arr.reshape(NCORES, nbp, NH * HID)
    q = blocks[:, :NB, :].astype(np.float32).reshape(N, NH * HID)
    sc = np.ascontiguousarray(blocks[:, NB:, :]).view(np.float32)
    np.multiply(q, sc.reshape(N, 1), out=q)
    return q
